# revision 11
# baseline (speedup 1.0000x reference)
"""Trainium2 Bass kernel for nn_NONLocal_Correlation (non-local block, B=2,
C=CI=256, N=8192).

Sharding: 8 cores = (batch b, query-chunk q) with b = core//4, q = core%4.
Each core computes out[b, :, q*2048:(q+1)*2048].

Per-core algorithm (all matmul operands float32r = fp32 bits, TF32-like PE
mode at full streaming rate):
  - x[b] is passed rolled by -q*2048 along n so the core's theta columns are
    always x_rot[:, :2048]; m-order permutation is irrelevant (softmax sums
    over m).
  - projections: phi (ci,m) full, g (m,ci) full, theta (ci,2048) chunk.
    theta/phi biases folded into the PSUM->SBUF epilogue (per-partition bias);
    g bias + w bias folded into z epilogue as zb = w_w @ g_b + w_b (since
    softmax rows sum to 1).
  - attention, scores transposed (m on partitions): per 512-wide n-chunk,
    accumulate over 64 m-blocks:
        f_T = phi_blk.T @ theta_chunk          (PSUM)
        E   = exp(f_T)                          (no max-subtraction needed:
                                                |f| < ~40 so fp32 exp is safe)
        y  += g_blk.T @ E                       (PSUM accumulate, ci=2x128)
        s  += ones.T @ E                        (row-sum broadcast to all
                                                partitions for free: cost is
                                                prop. to N only)
    then y_norm = y * recip(s); z = w_wT.T @ y_norm + zb.
  - BatchNorm (training stats over (b, n)): per-core partial sum/sumsq per
    channel, 2KB AllReduce across all 8 cores, then per-partition affine.
"""

import numpy as np

import concourse.bacc as bacc
import concourse.mybir as mybir
import concourse.tile as tile
from concourse.bass_utils import run_bass_kernel_spmd

B, C, N, CI = 2, 256, 8192, 256
CHUNK = N // 4            # 2048 query rows per core
W = 512                   # moving free-dim width
MB = N // 128             # 64 m-blocks
SW = 1024                 # x strip width for projections
EPS = 1e-5
NCORES = 8

F32 = mybir.dt.float32
F32R = mybir.dt.float32r
AF = mybir.ActivationFunctionType
AX = mybir.AxisListType


def build_body(nc, tc, pools, tensors):
    """Emit one full per-core computation. Separated so timing harnesses can
    replicate the body."""
    wp, xp, pp, ep, sp, psf, psa, psz, dp = pools
    x, wts, consts, out, ones_in = tensors

    # --- weights / constants ---
    w_sb = {}
    for p in ("th", "ph", "g", "ww"):
        for hi in (0, 1):
            t = wp.tile([128, CI], F32R, name=f"w_{p}{hi}", tag=f"w_{p}{hi}")
            nc.sync.dma_start(t[:], wts[p][hi])
            w_sb[p, hi] = t
    cst = wp.tile([128, 10], F32, name="cst", tag="cst")
    nc.sync.dma_start(cst[:], consts[:])
    ones = wp.tile([128, 128], F32R, name="ones", tag="ones")
    nc.sync.dma_start(ones[:], ones_in[:])

    # --- persistent activations ---
    phi = [pp.tile([128, N], F32R, name=f"phi{hi}", tag=f"phi{hi}") for hi in (0, 1)]
    gsb = pp.tile([128, MB * CI], F32R, name="gsb", tag="gsb")
    th = [pp.tile([128, CHUNK], F32R, name=f"th{hi}", tag=f"th{hi}") for hi in (0, 1)]
    zsb = [pp.tile([128, CHUNK], F32, name=f"z{hi}", tag=f"z{hi}") for hi in (0, 1)]
    # per-(nq,hi) BN partials: cols 0-7 sum (hi*4+nq), cols 8-15 sumsq
    parts = sp.tile([128, 16], F32, name="parts", tag="parts")

    # --- projections, x streamed in strips ---
    for s in range(N // SW):
        xs = []
        for hi in (0, 1):
            t = xp.tile([128, SW], F32R, name=f"xs{hi}", tag=f"xs{hi}")
            nc.sync.dma_start(t[:], x[hi * 128:(hi + 1) * 128, s * SW:(s + 1) * SW])
            xs.append(t)
        for ho in (0, 1):
            for sub in range(SW // W):
                sl = slice(sub * W, (sub + 1) * W)
                osl = slice(s * SW + sub * W, s * SW + (sub + 1) * W)
                fp = psf.tile([128, W], F32, name="mm_ps", tag="mm")
                nc.tensor.matmul(fp[:], w_sb["ph", 0][:, ho * 128:(ho + 1) * 128],
                                 xs[0][:, sl], start=True, stop=False)
                nc.tensor.matmul(fp[:], w_sb["ph", 1][:, ho * 128:(ho + 1) * 128],
                                 xs[1][:, sl], start=False, stop=True)
                nc.scalar.activation(phi[ho][:, osl], fp[:], AF.Identity,
                                     bias=cst[:, 2 + ho:3 + ho])
                if s * SW < CHUNK:  # theta only needs the first 2048 columns
                    tp = psf.tile([128, W], F32, name="mm_ps2", tag="mm")
                    nc.tensor.matmul(tp[:], w_sb["th", 0][:, ho * 128:(ho + 1) * 128],
                                     xs[0][:, sl], start=True, stop=False)
                    nc.tensor.matmul(tp[:], w_sb["th", 1][:, ho * 128:(ho + 1) * 128],
                                     xs[1][:, sl], start=False, stop=True)
                    nc.scalar.activation(th[ho][:, osl], tp[:], AF.Identity,
                                         bias=cst[:, 0 + ho:1 + ho])
        for blk in range(SW // 128):
            mb = s * (SW // 128) + blk
            bsl = slice(blk * 128, (blk + 1) * 128)
            gp = psf.tile([128, CI], F32, name="g_ps", tag="mm")
            nc.tensor.matmul(gp[:], xs[0][:, bsl], w_sb["g", 0][:],
                             start=True, stop=False)
            nc.tensor.matmul(gp[:], xs[1][:, bsl], w_sb["g", 1][:],
                             start=False, stop=True)
            nc.vector.tensor_copy(gsb[:, mb * CI:(mb + 1) * CI], gp[:])

    # --- attention + z, per 512-wide n-chunk ---
    for nq in range(CHUNK // W):
        nsl = slice(nq * W, (nq + 1) * W)
        yps = [psa.tile([128, W], F32, name=f"y_ps{hi}", tag=f"y_ps{hi}")
               for hi in (0, 1)]
        sps = psa.tile([128, W], F32, name="s_ps", tag="s_ps")
        for mb in range(MB):
            msl = slice(mb * 128, (mb + 1) * 128)
            fp = psf.tile([128, W], F32, name="f_ps", tag="mm")
            nc.tensor.matmul(fp[:], phi[0][:, msl], th[0][:, nsl],
                             start=True, stop=False)
            nc.tensor.matmul(fp[:], phi[1][:, msl], th[1][:, nsl],
                             start=False, stop=True)
            E = ep.tile([128, W], F32R, name="E", tag="E")
            nc.scalar.activation(E[:], fp[:], AF.Exp)
            st, fin = (mb == 0), (mb == MB - 1)
            nc.tensor.matmul(yps[0][:], gsb[:, mb * CI:mb * CI + 128],
                             E[:], start=st, stop=fin)
            nc.tensor.matmul(yps[1][:], gsb[:, mb * CI + 128:(mb + 1) * CI],
                             E[:], start=st, stop=fin)
            nc.tensor.matmul(sps[:], ones[:], E[:], start=st, stop=fin)
        rec = sp.tile([128, W], F32, name="rec", tag="rec", bufs=2)
        nc.vector.reciprocal(rec[:], sps[:])
        ysb = [sp.tile([128, W], F32R, name=f"ysb{hi}", tag=f"ysb{hi}", bufs=2)
               for hi in (0, 1)]
        for hi in (0, 1):
            nc.vector.tensor_mul(ysb[hi][:], yps[hi][:], rec[:])
        for hi in (0, 1):
            zp = psz.tile([128, W], F32, name=f"z_ps{hi}", tag="z_ps")
            nc.tensor.matmul(zp[:], w_sb["ww", 0][:, hi * 128:(hi + 1) * 128],
                             ysb[0][:], start=True, stop=False)
            nc.tensor.matmul(zp[:], w_sb["ww", 1][:, hi * 128:(hi + 1) * 128],
                             ysb[1][:], start=False, stop=True)
            nc.scalar.activation(zsb[hi][:, nsl], zp[:], AF.Identity,
                                 bias=cst[:, 4 + hi:5 + hi])
            col = hi * 4 + nq
            nc.vector.reduce_sum(parts[:, col:col + 1], zsb[hi][:, nsl], axis=AX.X)
            sq = sp.tile([128, W], F32, name="sq_scr", tag="sq_scr", bufs=2)
            nc.scalar.activation(sq[:], zsb[hi][:, nsl], AF.Square,
                                 accum_out=parts[:, 8 + col:9 + col])

    # --- BN stats: reduce partials, AllReduce, affine ---
    stats = sp.tile([128, 4], F32, name="stats", tag="stats")
    for j in range(4):
        nc.vector.reduce_sum(stats[:, j:j + 1], parts[:, j * 4:(j + 1) * 4], axis=AX.X)
    ar_in = dp.tile([128, 4], F32, name="ar_in", tag="ar_in")
    ar_out = dp.tile([128, 4], F32, name="ar_out", tag="ar_out",
                     addr_space="Shared")
    nc.gpsimd.dma_start(ar_in[:], stats[:])
    nc.gpsimd.collective_compute(
        "AllReduce", mybir.AluOpType.add,
        replica_groups=[list(range(NCORES))],
        ins=[ar_in.opt()], outs=[ar_out.opt()],
    )
    statsg = sp.tile([128, 4], F32, name="statsg", tag="statsg")
    nc.gpsimd.dma_start(statsg[:], ar_out[:])

    inv_cnt = 1.0 / (B * N)
    mean = sp.tile([128, 2], F32, name="mean", tag="mean")
    nc.scalar.mul(mean[:], statsg[:, 0:2], inv_cnt)
    ex2 = sp.tile([128, 2], F32, name="ex2", tag="ex2")
    nc.scalar.mul(ex2[:], statsg[:, 2:4], inv_cnt)
    msq = sp.tile([128, 2], F32, name="msq", tag="msq")
    nc.vector.tensor_mul(msq[:], mean[:], mean[:])
    var = sp.tile([128, 2], F32, name="var", tag="var")
    nc.vector.tensor_sub(var[:], ex2[:], msq[:])
    eps_t = sp.tile([128, 1], F32, name="eps_t", tag="eps_t")
    nc.vector.memset(eps_t[:], EPS)
    sd = sp.tile([128, 2], F32, name="sd", tag="sd")
    nc.scalar.activation(sd[:], var[:], AF.Sqrt, bias=eps_t[:])
    rinv = sp.tile([128, 2], F32, name="rinv", tag="rinv")
    nc.vector.reciprocal(rinv[:], sd[:])
    scl = sp.tile([128, 2], F32, name="scl", tag="scl")
    nc.vector.tensor_mul(scl[:], cst[:, 6:8], rinv[:])
    mscl = sp.tile([128, 2], F32, name="mscl", tag="mscl")
    nc.vector.tensor_mul(mscl[:], mean[:], scl[:])
    shf = sp.tile([128, 2], F32, name="shf", tag="shf")
    nc.vector.tensor_sub(shf[:], cst[:, 8:10], mscl[:])

    for hi in (0, 1):
        nc.scalar.activation(zsb[hi][:], zsb[hi][:], AF.Identity,
                             bias=shf[:, hi:hi + 1], scale=scl[:, hi:hi + 1])
        nc.sync.dma_start(out[hi * 128:(hi + 1) * 128, :], zsb[hi][:])


def build_nc(n_bodies=1):
    nc = bacc.Bacc("TRN2", target_bir_lowering=False, debug=False,
                   num_devices=NCORES)
    x = nc.dram_tensor("x", [C, N], F32R, kind="ExternalInput")
    wts = {p: nc.dram_tensor(f"w_{p}", [2, 128, CI], F32R, kind="ExternalInput")
           for p in ("th", "ph", "g", "ww")}
    consts = nc.dram_tensor("consts", [128, 10], F32, kind="ExternalInput")
    ones_in = nc.dram_tensor("ones_in", [128, 128], F32R, kind="ExternalInput")
    out = nc.dram_tensor("out", [CI, CHUNK], F32, kind="ExternalOutput")

    with tile.TileContext(nc) as tc:
        with (
            tc.tile_pool(name="wp", bufs=1) as wp,
            tc.tile_pool(name="xp", bufs=2) as xp,
            tc.tile_pool(name="pp", bufs=1) as pp,
            tc.tile_pool(name="ep", bufs=3) as ep,
            tc.tile_pool(name="sp", bufs=1) as sp,
            tc.tile_pool(name="psf", bufs=3, space="PSUM") as psf,
            tc.tile_pool(name="psa", bufs=1, space="PSUM") as psa,
            tc.tile_pool(name="psz", bufs=2, space="PSUM") as psz,
            tc.tile_pool(name="dp", bufs=1, space="DRAM") as dp,
        ):
            pools = (wp, xp, pp, ep, sp, psf, psa, psz, dp)
            tensors = (x, wts, consts, out, ones_in)
            for _ in range(n_bodies):
                build_body(nc, tc, pools, tensors)
    nc.compile()
    return nc


def make_in_maps(inputs):
    x = np.asarray(inputs["x"], np.float32)
    wT = {
        "th": np.ascontiguousarray(np.asarray(inputs["theta_w"], np.float32).T
                                   .reshape(2, 128, CI)),
        "ph": np.ascontiguousarray(np.asarray(inputs["phi_w"], np.float32).T
                                   .reshape(2, 128, CI)),
        "g": np.ascontiguousarray(np.asarray(inputs["g_w"], np.float32).T
                                  .reshape(2, 128, CI)),
        "ww": np.ascontiguousarray(np.asarray(inputs["w_w"], np.float32).T
                                   .reshape(2, 128, CI)),
    }
    zb = (np.asarray(inputs["w_w"], np.float32) @ np.asarray(inputs["g_b"], np.float32)
          + np.asarray(inputs["w_b"], np.float32))
    consts = np.zeros((128, 10), np.float32)
    for j, v in enumerate((inputs["theta_b"], inputs["phi_b"], zb,
                           inputs["bn_gamma"], inputs["bn_beta"])):
        v = np.asarray(v, np.float32)
        consts[:, 2 * j] = v[:128]
        consts[:, 2 * j + 1] = v[128:]
    in_maps = []
    for k in range(NCORES):
        b, q = divmod(k, 4)
        xb = np.roll(x[b], -q * CHUNK, axis=1)
        in_maps.append({
            "x": np.ascontiguousarray(xb),
            "w_th": wT["th"], "w_ph": wT["ph"], "w_g": wT["g"], "w_ww": wT["ww"],
            "consts": consts, "ones_in": np.ones((128, 128), np.float32),
        })
    return in_maps


def assemble(results):
    out = np.empty((B, CI, N), np.float32)
    for k in range(NCORES):
        b, q = divmod(k, 4)
        out[b, :, q * CHUNK:(q + 1) * CHUNK] = results[k]["out"]
    return out


_NC_CACHE = {}


def kernel(**inputs) -> np.ndarray:
    if "nc" not in _NC_CACHE:
        _NC_CACHE["nc"] = build_nc()
    nc = _NC_CACHE["nc"]
    in_maps = make_in_maps(inputs)
    res = run_bass_kernel_spmd(nc, in_maps, list(range(NCORES)))
    return assemble(res.results)


# revision 38
# speedup vs baseline: 425.7496x; 425.7496x over previous
"""Trainium2 Bass kernel for nn_NONLocal_Correlation (non-local block, B=2,
C=CI=256, N=8192).

Sharding: 8 cores = (batch b, query-chunk q) with b = core//4, q = core%4.
Each core computes out[b, :, q*2048:(q+1)*2048].

Per-core algorithm (all matmul operands float32r = fp32 bits, TF32-like PE
mode at full streaming rate):
  - x[b] is passed rolled by -q*2048 along n so the core's theta columns are
    always x_rot[:, :2048]; m-order permutation is irrelevant (softmax sums
    over m).
  - projections: phi (ci,m) full, g (m,ci) full, theta (ci,2048) chunk.
    theta/phi biases folded into the PSUM->SBUF epilogue (per-partition bias);
    g bias + w bias folded into z epilogue as zb = w_w @ g_b + w_b (since
    softmax rows sum to 1).  Projection strips are interleaved into the first
    attention chunk's m-loop so the PE never waits.
  - attention, scores transposed (m on partitions): per 512-wide n-chunk,
    accumulate over 64 m-blocks:
        f_T = phi_blk.T @ theta_chunk          (PSUM)
        E   = exp(f_T)                          (no max-subtraction needed:
                                                |f| < ~40 so fp32 exp is safe)
        y  += g_blk.T @ E                       (PSUM accumulate, ci=2x128)
        s_acc += E                              (DVE; softmax denominator)
    then partition tree-reduce s_acc, reciprocal, gpsimd partition-broadcast,
    y_norm = y * rec; z = w_wT.T @ y_norm + zb.
  - BatchNorm (training stats over (b, n)): per-core partial sum/sumsq per
    channel, 2KB AllReduce across all 8 cores, then per-partition affine.
"""

import numpy as np

import concourse.bacc as bacc
import concourse.mybir as mybir
import concourse.tile as tile
from concourse.bass_utils import run_bass_kernel_spmd

B, C, N, CI = 2, 256, 8192, 256
CHUNK = N // 4            # 2048 query rows per core
W = 512                   # moving free-dim width
MB = N // 128             # 64 m-blocks
SW = 512                  # x strip width for projections
MB_PER_STRIP = SW // 128  # 4
EPS = 1e-5
NCORES = 8

F32 = mybir.dt.float32
F32R = mybir.dt.float32r
AF = mybir.ActivationFunctionType
AX = mybir.AxisListType


def build_body(nc, tc, pools, tensors):
    """Emit one full per-core computation. Separated so timing harnesses can
    replicate the body."""
    wp, xp, pp, ep, sp, psf, psa, psz, dp = pools
    x, wts, consts, out, ones_in = tensors

    # --- first x strip prefetch (ahead of weights in the sync DMA queue) ---
    xs0 = []
    for hi in (0, 1):
        t = xp.tile([128, SW], F32R, name=f"xs{hi}", tag=f"xs{hi}")
        nc.sync.dma_start(t[:], x[hi * 128:(hi + 1) * 128, 0:SW])
        xs0.append(t)

    # --- weights / constants (phi weights go on the scalar queue so they
    # don't queue behind the x strip on sync; theta next on sync) ---
    w_sb = {}
    for p in ("ph", "th", "g", "ww"):
        for hi in (0, 1):
            t = wp.tile([128, CI], F32R, name=f"w_{p}{hi}", tag=f"w_{p}{hi}")
            (nc.scalar if p == "ph" else nc.sync).dma_start(t[:], wts[p][hi])
            w_sb[p, hi] = t
    cst = wp.tile([128, 10], F32, name="cst", tag="cst")
    nc.scalar.dma_start(cst[:], consts[:])
    ones = wp.tile([128, 128], F32R, name="ones", tag="ones")
    nc.scalar.dma_start(ones[:], ones_in[:])

    # --- persistent activations ---
    phi = [pp.tile([128, N], F32R, name=f"phi{hi}", tag=f"phi{hi}") for hi in (0, 1)]
    gsb = pp.tile([128, MB * CI], F32R, name="gsb", tag="gsb")
    th = [pp.tile([128, CHUNK], F32R, name=f"th{hi}", tag=f"th{hi}") for hi in (0, 1)]
    zsb = [pp.tile([128, CHUNK], F32, name=f"z{hi}", tag=f"z{hi}") for hi in (0, 1)]
    # per-(nq,hi) BN partials: cols hi*4+nq sum, 8 + hi*4+nq sumsq
    parts = sp.tile([128, 16], F32, name="parts", tag="parts")

    def proj_strip(s, xs=None):
        if xs is None:
            xs = []
            for hi in (0, 1):
                t = xp.tile([128, SW], F32R, name=f"xs{hi}", tag=f"xs{hi}")
                nc.sync.dma_start(t[:], x[hi * 128:(hi + 1) * 128,
                                          s * SW:(s + 1) * SW])
                xs.append(t)
        for ho in (0, 1):
            for sub in range(SW // W):
                sl = slice(sub * W, (sub + 1) * W)
                osl = slice(s * SW + sub * W, s * SW + (sub + 1) * W)
                fp = psf.tile([128, W], F32, name="mm_ps", tag="mm")
                nc.tensor.matmul(fp[:], w_sb["ph", 0][:, ho * 128:(ho + 1) * 128],
                                 xs[0][:, sl], start=True, stop=False)
                nc.tensor.matmul(fp[:], w_sb["ph", 1][:, ho * 128:(ho + 1) * 128],
                                 xs[1][:, sl], start=False, stop=True)
                nc.scalar.activation(phi[ho][:, osl], fp[:], AF.Identity,
                                     bias=cst[:, 2 + ho:3 + ho])
                if s * SW < CHUNK:  # theta only needs the first 2048 columns
                    tp = psf.tile([128, W], F32, name="mm_ps2", tag="mm")
                    nc.tensor.matmul(tp[:], w_sb["th", 0][:, ho * 128:(ho + 1) * 128],
                                     xs[0][:, sl], start=True, stop=False)
                    nc.tensor.matmul(tp[:], w_sb["th", 1][:, ho * 128:(ho + 1) * 128],
                                     xs[1][:, sl], start=False, stop=True)
                    nc.scalar.activation(th[ho][:, osl], tp[:], AF.Identity,
                                         bias=cst[:, 0 + ho:1 + ho])
        for blk in range(MB_PER_STRIP):
            mb = s * MB_PER_STRIP + blk
            bsl = slice(blk * 128, (blk + 1) * 128)
            gp = psf.tile([128, CI], F32, name="g_ps", tag="mm")
            nc.tensor.matmul(gp[:], xs[0][:, bsl], w_sb["g", 0][:],
                             start=True, stop=False)
            nc.tensor.matmul(gp[:], xs[1][:, bsl], w_sb["g", 1][:],
                             start=False, stop=True)
            nc.vector.tensor_copy(gsb[:, mb * CI:(mb + 1) * CI], gp[:])

    # attention state per n-chunk (created by att_begin)
    att = {}

    def att_begin(nq):
        att[nq] = {
            "yps": [psa.tile([128, W], F32, name=f"y_ps{hi}", tag=f"y_ps{hi}",
                             bufs=2) for hi in (0, 1)],
            "sacc": sp.tile([128, W], F32R, name="s_acc", tag="s_acc", bufs=2),
        }

    def att_segment(nq, mb_lo, mb_hi):
        nsl = slice(nq * W, (nq + 1) * W)
        yps, sacc = att[nq]["yps"], att[nq]["sacc"]
        for mb in range(mb_lo, mb_hi):
            msl = slice(mb * 128, (mb + 1) * 128)
            fp = psf.tile([128, W], F32, name="f_ps", tag="mm")
            nc.tensor.matmul(fp[:], phi[0][:, msl], th[0][:, nsl],
                             start=True, stop=False)
            nc.tensor.matmul(fp[:], phi[1][:, msl], th[1][:, nsl],
                             start=False, stop=True)
            E = ep.tile([128, W], F32R, name="E", tag="E")
            nc.scalar.activation(E[:], fp[:], AF.Exp)
            st, fin = (mb == 0), (mb == MB - 1)
            nc.tensor.matmul(yps[0][:], gsb[:, mb * CI:mb * CI + 128],
                             E[:], start=st, stop=fin)
            nc.tensor.matmul(yps[1][:], gsb[:, mb * CI + 128:(mb + 1) * CI],
                             E[:], start=st, stop=fin)
            if mb == 0:
                nc.vector.tensor_copy(sacc[:], E[:])
            else:
                nc.vector.tensor_add(sacc[:], sacc[:], E[:])

    def tail_a(nq):
        """Chunk tail part 1: copy unnormalized y to SBUF (gates PE's z
        matmuls), and reduce+broadcast the softmax denominators with a single
        ones-matmul (rec is consumed only at the z epilogue, off PE's critical
        path since z*rec+zb == (w@y_unnorm)*rec+zb)."""
        yps, sacc = att[nq]["yps"], att[nq]["sacc"]
        ysb = [sp.tile([128, W], F32R, name=f"ysb{hi}", tag=f"ysb{hi}", bufs=2)
               for hi in (0, 1)]
        nc.vector.tensor_copy(ysb[0][:], yps[0][:])
        nc.scalar.activation(ysb[1][:], yps[1][:], AF.Identity)
        s_red = psf.tile([128, W], F32, name="s_red", tag="mm")
        nc.tensor.matmul(s_red[:], ones[:], sacc[:], start=True, stop=True)
        rec = sp.tile([128, W], F32, name="rec", tag="rec", bufs=2)
        nc.vector.reciprocal(rec[:], s_red[:])
        att[nq]["ysb"] = ysb
        att[nq]["rec"] = rec

    def tail_z(nq):
        """PE part of the chunk tail: z matmuls + epilogue + BN partials."""
        nsl = slice(nq * W, (nq + 1) * W)
        ysb, rec = att[nq]["ysb"], att[nq]["rec"]
        for hi in (0, 1):
            zp = psf.tile([128, W], F32, name=f"z_ps{hi}", tag="mm")
            nc.tensor.matmul(zp[:], w_sb["ww", 0][:, hi * 128:(hi + 1) * 128],
                             ysb[0][:], start=True, stop=False)
            nc.tensor.matmul(zp[:], w_sb["ww", 1][:, hi * 128:(hi + 1) * 128],
                             ysb[1][:], start=False, stop=True)
            nc.vector.tensor_mul(zsb[hi][:, nsl], zp[:], rec[:])
            nc.scalar.activation(zsb[hi][:, nsl], zsb[hi][:, nsl], AF.Identity,
                                 bias=cst[:, 4 + hi:5 + hi])
            col = hi * 4 + nq
            nc.vector.reduce_sum(parts[:, col:col + 1], zsb[hi][:, nsl], axis=AX.X)
            sq = sp.tile([128, W], F32, name="sq_scr", tag="sq_scr", bufs=1)
            nc.scalar.activation(sq[:], zsb[hi][:, nsl], AF.Square,
                                 accum_out=parts[:, 8 + col:9 + col])

    # --- emission: interleave projections into attention chunk 0, and each
    # chunk's z-tail into the next chunk's m-loop (PE is in-order) ---
    NQ = CHUNK // W
    proj_strip(0, xs=xs0)  # theta cols for chunk 0 + first phi/g blocks
    att_begin(0)
    att_segment(0, 0, MB_PER_STRIP)
    for s in range(1, N // SW):
        proj_strip(s)
        att_segment(0, s * MB_PER_STRIP, (s + 1) * MB_PER_STRIP)
    tail_a(0)
    for nq in range(1, NQ):
        att_begin(nq)
        att_segment(nq, 0, 4)
        tail_z(nq - 1)
        att_segment(nq, 4, MB)
        tail_a(nq)
    tail_z(NQ - 1)

    # --- BN stats: reduce partials, AllReduce, affine ---
    stats = sp.tile([128, 4], F32, name="stats", tag="stats")
    nc.vector.reduce_sum(stats[:], parts[:].rearrange("p (g c) -> p g c", c=4),
                         axis=AX.X)
    ar_in = dp.tile([128, 4], F32, name="ar_in", tag="ar_in")
    ar_out = dp.tile([128, 4], F32, name="ar_out", tag="ar_out",
                     addr_space="Shared")
    nc.sync.dma_start(ar_in[:], stats[:])
    nc.gpsimd.collective_compute(
        "AllReduce", mybir.AluOpType.add,
        replica_groups=[list(range(NCORES))],
        ins=[ar_in.opt()], outs=[ar_out.opt()],
    )
    statsg = sp.tile([128, 4], F32, name="statsg", tag="statsg")
    nc.sync.dma_start(statsg[:], ar_out[:])

    inv_cnt = 1.0 / (B * N)
    moments = sp.tile([128, 4], F32, name="moments", tag="moments")
    nc.scalar.mul(moments[:], statsg[:], inv_cnt)
    mean, ex2 = moments[:, 0:2], moments[:, 2:4]
    msq = sp.tile([128, 2], F32, name="msq", tag="msq")
    nc.vector.tensor_mul(msq[:], mean, mean)
    var = sp.tile([128, 2], F32, name="var", tag="var")
    nc.vector.tensor_sub(var[:], ex2, msq[:])
    eps_t = sp.tile([128, 1], F32, name="eps_t", tag="eps_t")
    nc.vector.memset(eps_t[:], EPS)
    sd = sp.tile([128, 2], F32, name="sd", tag="sd")
    nc.scalar.activation(sd[:], var[:], AF.Sqrt, bias=eps_t[:])
    rinv = sp.tile([128, 2], F32, name="rinv", tag="rinv")
    nc.vector.reciprocal(rinv[:], sd[:])
    scl = sp.tile([128, 2], F32, name="scl", tag="scl")
    nc.vector.tensor_mul(scl[:], cst[:, 6:8], rinv[:])
    mscl = sp.tile([128, 2], F32, name="mscl", tag="mscl")
    nc.vector.tensor_mul(mscl[:], mean, scl[:])
    shf = sp.tile([128, 2], F32, name="shf", tag="shf")
    nc.vector.tensor_sub(shf[:], cst[:, 8:10], mscl[:])

    # final affine + writeback, split across engines and pipelined in halves
    HALF = CHUNK // 2
    for j in (0, 1):
        jsl = slice(j * HALF, (j + 1) * HALF)
        nc.vector.tensor_scalar(zsb[0][:, jsl], zsb[0][:, jsl],
                                scl[:, 0:1], shf[:, 0:1],
                                mybir.AluOpType.mult, mybir.AluOpType.add)
        nc.scalar.activation(zsb[1][:, jsl], zsb[1][:, jsl], AF.Identity,
                             bias=shf[:, 1:2], scale=scl[:, 1:2])
        nc.sync.dma_start(out[0:128, jsl], zsb[0][:, jsl])
        nc.scalar.dma_start(out[128:256, jsl], zsb[1][:, jsl])


def build_nc(n_bodies=1):
    nc = bacc.Bacc("TRN2", target_bir_lowering=False, debug=False,
                   num_devices=NCORES)
    x = nc.dram_tensor("x", [C, N], F32R, kind="ExternalInput")
    wts = {p: nc.dram_tensor(f"w_{p}", [2, 128, CI], F32R, kind="ExternalInput")
           for p in ("th", "ph", "g", "ww")}
    consts = nc.dram_tensor("consts", [128, 10], F32, kind="ExternalInput")
    ones_in = nc.dram_tensor("ones_in", [128, 128], F32R, kind="ExternalInput")
    out = nc.dram_tensor("out", [CI, CHUNK], F32, kind="ExternalOutput")

    with tile.TileContext(nc) as tc:
        with (
            tc.tile_pool(name="wp", bufs=1) as wp,
            tc.tile_pool(name="xp", bufs=2) as xp,
            tc.tile_pool(name="pp", bufs=1) as pp,
            tc.tile_pool(name="ep", bufs=3) as ep,
            tc.tile_pool(name="sp", bufs=1) as sp,
            tc.tile_pool(name="psf", bufs=4, space="PSUM") as psf,
            tc.tile_pool(name="psa", bufs=1, space="PSUM") as psa,
            tc.tile_pool(name="dp", bufs=1, space="DRAM") as dp,
        ):
            pools = (wp, xp, pp, ep, sp, psf, psa, None, dp)
            tensors = (x, wts, consts, out, ones_in)
            for _ in range(n_bodies):
                build_body(nc, tc, pools, tensors)
    nc.compile()
    return nc


def make_in_maps(inputs):
    x = np.asarray(inputs["x"], np.float32)
    wT = {
        "th": np.ascontiguousarray(np.asarray(inputs["theta_w"], np.float32).T
                                   .reshape(2, 128, CI)),
        "ph": np.ascontiguousarray(np.asarray(inputs["phi_w"], np.float32).T
                                   .reshape(2, 128, CI)),
        "g": np.ascontiguousarray(np.asarray(inputs["g_w"], np.float32).T
                                  .reshape(2, 128, CI)),
        "ww": np.ascontiguousarray(np.asarray(inputs["w_w"], np.float32).T
                                   .reshape(2, 128, CI)),
    }
    zb = (np.asarray(inputs["w_w"], np.float32) @ np.asarray(inputs["g_b"], np.float32)
          + np.asarray(inputs["w_b"], np.float32))
    consts = np.zeros((128, 10), np.float32)
    for j, v in enumerate((inputs["theta_b"], inputs["phi_b"], zb,
                           inputs["bn_gamma"], inputs["bn_beta"])):
        v = np.asarray(v, np.float32)
        consts[:, 2 * j] = v[:128]
        consts[:, 2 * j + 1] = v[128:]
    in_maps = []
    for k in range(NCORES):
        b, q = divmod(k, 4)
        xb = np.roll(x[b], -q * CHUNK, axis=1)
        in_maps.append({
            "x": np.ascontiguousarray(xb),
            "w_th": wT["th"], "w_ph": wT["ph"], "w_g": wT["g"], "w_ww": wT["ww"],
            "consts": consts, "ones_in": np.ones((128, 128), np.float32),
        })
    return in_maps


def assemble(results):
    out = np.empty((B, CI, N), np.float32)
    for k in range(NCORES):
        b, q = divmod(k, 4)
        out[b, :, q * CHUNK:(q + 1) * CHUNK] = results[k]["out"]
    return out


_NC_CACHE = {}


def kernel(**inputs) -> np.ndarray:
    if "nc" not in _NC_CACHE:
        _NC_CACHE["nc"] = build_nc()
    nc = _NC_CACHE["nc"]
    in_maps = make_in_maps(inputs)
    res = run_bass_kernel_spmd(nc, in_maps, list(range(NCORES)))
    return assemble(res.results)
